# revision 19
# baseline (speedup 1.0000x reference)
"""ALCDEF Temporal GNN (gnn_message_passing) on 8 TRN2 NeuronCores.

Self-contained: takes FULL unsharded inputs, returns FULL [64,1] output.

Strategy (all shapes hardcoded for N=50000, E=800000, F_IN=16, H=128, L=3, G=64):
- Nodes dst-sharded across 8 cores (6250 each, padded to 6656 local slots;
  local node l = w*128 + p for window w, partition p).
- Per layer each core computes its shard of the message table
  m~[n] = dis_n * (h @ W_l) (node-major bf16 rows, row r = c*6656 + p*52 + w)
  and AllGathers it so every core holds the full table in DRAM.
- Edge aggregation: SWDGE dma_gather pulls 1024 message rows per call
  (edge-major msgT [128 edges, 128 feat]); one-hot S matrices (streamed
  bf16, built on host) scatter-add them into per-window PSUM banks via
  TensorE matmuls; self-loops are injected with an identity matmul from
  the local node-major m~ copy; the GCN bias rides a dedicated table row
  with S value 1/dis_d.
- LayerNorm is computed node-major per PSUM bank with an exact per-node
  eps correction (eps_d = eps*deg_d) that makes the dis_d folding exact.
- Mean-pool via per-window matmuls with a host-built B matrix, AllReduce,
  then the small head MLP (softplus composed from Relu/Exp/Ln) on every
  core; core 0's output is returned.
"""
import sys
sys.path.insert(0, "/opt/trn_rl_repo")
import numpy as np
import ml_dtypes
from contextlib import ExitStack

bf16 = ml_dtypes.bfloat16
f32 = np.float32

N, E, F_IN, H, L, G = 50000, 800000, 16, 128, 3, 64
LN_EPS = 1e-5
P = 8
SH = 6656
W = 128
NW = SH // W          # 52
BANKS = NW // 4       # 13
SHARD = 6250
HALF = 4 * SH         # 26624 table rows per half
BATCH = 8             # chunks per gather call (1024 idxs)
NQ = 1                # SWDGE queues (desc-gen pipelines across Q7 core pairs)
NT512 = SH // 512     # 13


def _row_of_local(l):
    w, p = l // W, l % W
    return p * NW + w


def build_host(inputs):
    edge_index = np.asarray(inputs["edge_index"])
    batch = np.asarray(inputs["batch"]).astype(np.int64)
    src_g = edge_index[0].astype(np.int64)
    dst_g = edge_index[1].astype(np.int64)

    deg = np.bincount(dst_g, minlength=N).astype(np.float64) + 1.0
    dis = (1.0 / np.sqrt(deg)).astype(np.float64)

    own = np.minimum(src_g // SHARD, P - 1)
    src_loc = src_g - own * SHARD
    src_row = own * SH + (src_loc % W) * NW + (src_loc // W)

    own_d = np.minimum(dst_g // SHARD, P - 1)
    d_loc = dst_g - own_d * SHARD
    win = d_loc // W
    half = (src_row >= HALF).astype(np.int64)

    BROW = _row_of_local(6250)
    ZROW = _row_of_local(6251)

    # one gather slot per distinct (window, half, src); S row holds all its
    # dst columns (values = edge multiplicity)
    slot_cnt = np.zeros((P, NW, 2), dtype=np.int64)
    for c in range(P):
        m = own_d == c
        for h in range(2):
            mm_ = m & (half == h)
            pairs = win[mm_] * (HALF + 1) + src_row[mm_] - h * HALF
            slot_cnt[c, :, h] = np.bincount(
                np.unique(pairs) // (HALF + 1), minlength=NW)
    slot_cnt[:, :, 0] += 1  # bias slot in half A
    chunks_max = np.ceil(slot_cnt / 128).astype(np.int64).max(axis=0)

    seq = []  # (bank, half, wi, start)
    for b in range(BANKS):
        for h in range(2):
            for wi in range(4):
                for k in range(chunks_max[4 * b + wi, h]):
                    seq.append((b, h, wi, k == 0 and h == 0))
    NCH = len(seq)
    struct = {"seq": seq, "NCH": NCH}

    per_core = []
    lc = np.asarray(inputs["lightcurve"], f32)
    counts = np.bincount(batch, minlength=G).astype(np.float64)
    for c in range(P):
        m = np.flatnonzero(own_d == c)
        e_half = half[m]
        e_win = win[m]
        e_row = src_row[m] - e_half * HALF
        e_col = d_loc[m] % W

        idx_blob = np.zeros((NCH, 128), dtype=np.int16)
        S = np.zeros((NCH, 128, W), dtype=f32)

        pos = {}
        for k, (b, h, wi, st) in enumerate(seq):
            if wi >= 0:
                pos.setdefault((4 * b + wi, h), []).append(k)
        for (w_, h), ks in pos.items():
            sel = np.flatnonzero((e_win == w_) & (e_half == h))
            rows_e = e_row[sel]
            cols_e = e_col[sel]
            uniq, inv = np.unique(rows_e, return_inverse=True)
            nslots = len(uniq)
            cap = len(ks) * 128
            bias_slot = nslots if h == 0 else -1
            total = nslots + (1 if h == 0 else 0)
            assert total <= cap, (w_, h, total, cap)
            slot_rows = np.full(cap, ZROW, dtype=np.int16)
            slot_rows[:nslots] = uniq.astype(np.int16)
            if h == 0:
                slot_rows[bias_slot] = BROW
            for j, k in enumerate(ks):
                idx_blob[k] = slot_rows[j * 128:(j + 1) * 128]
            kk = np.array(ks)[inv // 128]
            np.add.at(S, (kk, inv % 128, cols_e), 1.0)
            if h == 0:
                kb = ks[bias_slot // 128]
                node0 = w_ * W
                g0 = c * SHARD
                nreal = min(W, SHARD - node0)
                if nreal > 0:
                    S[kb, bias_slot % 128, :nreal] = \
                        1.0 / dis[g0 + node0:g0 + node0 + nreal]

        wrapped = idx_blob.reshape(NCH, 8, 16).transpose(2, 0, 1).reshape(16, NCH * 8)
        wrapped = np.tile(wrapped, (8, 1)).astype(np.int16)

        g0 = c * SHARD
        dis_loc = np.zeros(SH, f32)
        dis_loc[:SHARD] = dis[g0:g0 + SHARD]
        eps_loc = np.full(SH, LN_EPS, f32)
        eps_loc[:SHARD] = (LN_EPS * deg[g0:g0 + SHARD]).astype(f32)
        dis_tile = dis_loc.reshape(NW, W).T.copy()
        eps_tile = eps_loc.reshape(NW, W).T.copy()

        Bp = np.zeros((SH, G), f32)
        bb = batch[g0:g0 + SHARD]
        Bp[np.arange(SHARD), bb] = (1.0 / np.maximum(counts[bb], 1.0)).astype(f32)
        B_tile = Bp.reshape(NW, W, G).transpose(1, 0, 2).copy()

        lcT = np.zeros((F_IN, SH), f32)
        lcT[:, :SHARD] = lc[g0:g0 + SHARD].T

        per_core.append({
            "idx": np.ascontiguousarray(wrapped),
            "S": np.ascontiguousarray(S.transpose(1, 0, 2)).astype(bf16),
            "dis_tile": dis_tile, "eps_tile": eps_tile,
            "B_tile": np.ascontiguousarray(B_tile),
            "lcT": lcT,
        })

    consts = {
        "enc_W1": np.asarray(inputs["enc_W1"], f32),
        "enc_b1": np.asarray(inputs["enc_b1"], f32).reshape(H, 1),
        "enc_W2": np.asarray(inputs["enc_W2"], f32),
        "enc_b2": np.asarray(inputs["enc_b2"], f32).reshape(H, 1),
        "convW": np.asarray(inputs["conv_W"], f32).astype(bf16),
        "conv_b_bf": np.asarray(inputs["conv_b"], f32).astype(bf16).reshape(L, 1, H),
        "g_rep": np.tile(np.asarray(inputs["ln_g"], f32)[:, None, :], (1, 128, 1)),
        "b_rep": np.tile(np.asarray(inputs["ln_b"], f32)[:, None, :], (1, 128, 1)),
        "ident_bf": np.eye(128, dtype=f32).astype(bf16),
        "ident_f32": np.eye(128, dtype=f32),
        "hW1": np.asarray(inputs["h_W1"], f32),
        "hb1": np.asarray(inputs["h_b1"], f32).reshape(64, 1),
        "hW2": np.asarray(inputs["h_W2"], f32),
        "hb2": np.asarray(inputs["h_b2"], f32).reshape(32, 1),
        "hW3": np.asarray(inputs["h_W3"], f32),
        "hb3": np.asarray(inputs["h_b3"], f32).reshape(1, 1),
    }
    return struct, per_core, consts


def build_nc(struct, nrep=1):
    import zlib
    nonce = (zlib.crc32((repr(struct["seq"]) + "v5" + str(nrep)).encode()) % 997) + 2
    import concourse.bass as bass
    import concourse.bacc as bacc
    import concourse.mybir as mybir
    import concourse.tile as tile
    from concourse.library_config import mlp as mlp_lib

    seq = struct["seq"]
    NCH = struct["NCH"]
    AF = mybir.ActivationFunctionType
    ALU = mybir.AluOpType
    DT = mybir.dt

    nc = bacc.Bacc("TRN2", debug=False, num_devices=P)
    dp = nc.declare_dram_parameter
    idx_ext = dp("idx", [128, NCH * 8], DT.int16, isOutput=False)
    S_ext = dp("S", [128, NCH, W], DT.bfloat16, isOutput=False)
    lcT_ext = dp("lcT", [F_IN, SH], DT.float32, isOutput=False)
    dis_ext = dp("dis_tile", [128, NW], DT.float32, isOutput=False)
    eps_ext = dp("eps_tile", [128, NW], DT.float32, isOutput=False)
    B_ext = dp("B_tile", [128, NW, G], DT.float32, isOutput=False)
    encW1_ext = dp("enc_W1", [F_IN, H], DT.float32, isOutput=False)
    encb1_ext = dp("enc_b1", [H, 1], DT.float32, isOutput=False)
    encW2_ext = dp("enc_W2", [H, H], DT.float32, isOutput=False)
    encb2_ext = dp("enc_b2", [H, 1], DT.float32, isOutput=False)
    convW_ext = dp("convW", [L, H, H], DT.bfloat16, isOutput=False)
    convb_ext = dp("conv_b_bf", [L, 1, H], DT.bfloat16, isOutput=False)
    grep_ext = dp("g_rep", [L, 128, H], DT.float32, isOutput=False)
    brep_ext = dp("b_rep", [L, 128, H], DT.float32, isOutput=False)
    identbf_ext = dp("ident_bf", [128, 128], DT.bfloat16, isOutput=False)
    identf_ext = dp("ident_f32", [128, 128], DT.float32, isOutput=False)
    hW1_ext = dp("hW1", [H, 64], DT.float32, isOutput=False)
    hb1_ext = dp("hb1", [64, 1], DT.float32, isOutput=False)
    hW2_ext = dp("hW2", [64, 32], DT.float32, isOutput=False)
    hb2_ext = dp("hb2", [32, 1], DT.float32, isOutput=False)
    hW3_ext = dp("hW3", [32, 1], DT.float32, isOutput=False)
    hb3_ext = dp("hb3", [1, 1], DT.float32, isOutput=False)
    out_ext = dp("out", [G, 1], DT.float32, isOutput=True)
    dp("nonce", [1, nonce], DT.float32, isOutput=False)

    mloc_th = nc.dram_tensor("mloc_i", [128, NW * H], DT.bfloat16)
    table_th = nc.dram_tensor("table_i", [P * SH, H], DT.bfloat16, addr_space="Shared")
    pool_loc_th = nc.dram_tensor("pool_loc_i", [G, H], DT.float32)
    pool_full_th = nc.dram_tensor("pool_full_i", [G, H], DT.float32, addr_space="Shared")

    with tile.TileContext(nc) as tc, ExitStack() as ctx:
        mloc_d = mloc_th.ap()
        table_d = table_th.ap()
        pool_loc_d = pool_loc_th.ap()
        pool_full_d = pool_full_th.ap()
        nc.gpsimd.load_library(mlp_lib)
        pers = ctx.enter_context(tc.tile_pool(name="pers", bufs=1))
        sb = ctx.enter_context(tc.tile_pool(name="sb", bufs=3))
        sstream = ctx.enter_context(tc.tile_pool(name="sstream", bufs=7))
        msgs = ctx.enter_context(tc.tile_pool(name="msgs", bufs=9))
        psA = ctx.enter_context(tc.tile_pool(name="psA", bufs=4, space="PSUM"))
        psB = ctx.enter_context(tc.tile_pool(name="psB", bufs=2, space="PSUM"))
        psC = ctx.enter_context(tc.tile_pool(name="psC", bufs=2, space="PSUM"))

        idx_t = pers.tile([128, NCH * 8], DT.int16, tag="idx")
        nc.sync.dma_start(idx_t[:], idx_ext[:, :])
        dis_t = pers.tile([128, NW], DT.float32, tag="dis")
        nc.sync.dma_start(dis_t[:], dis_ext[:, :])
        eps_t = pers.tile([128, NW], DT.float32, tag="eps")
        nc.sync.dma_start(eps_t[:], eps_ext[:, :])
        B_t = pers.tile([128, NW, G], DT.float32, tag="B")
        nc.sync.dma_start(B_t[:], B_ext[:, :, :])
        lcT_t = pers.tile([F_IN, SH], DT.float32, tag="lcT")
        nc.sync.dma_start(lcT_t[:], lcT_ext[:, :])
        encW1_t = pers.tile([F_IN, H], DT.float32, tag="encW1")
        nc.sync.dma_start(encW1_t[:], encW1_ext[:, :])
        encb1_t = pers.tile([H, 1], DT.float32, tag="encb1")
        nc.sync.dma_start(encb1_t[:], encb1_ext[:, :])
        encW2_t = pers.tile([H, H], DT.float32, tag="encW2")
        nc.sync.dma_start(encW2_t[:], encW2_ext[:, :])
        encb2_t = pers.tile([H, 1], DT.float32, tag="encb2")
        nc.sync.dma_start(encb2_t[:], encb2_ext[:, :])
        convW_t = pers.tile([H, L, H], DT.bfloat16, tag="convW")
        nc.sync.dma_start(convW_t[:], convW_ext.ap().rearrange("l a b -> a l b"))
        convb_t = pers.tile([1, L, H], DT.bfloat16, tag="convb")
        nc.sync.dma_start(convb_t[:], convb_ext.ap().rearrange("l a b -> a l b"))
        grep_t = pers.tile([128, L, H], DT.float32, tag="grep")
        nc.sync.dma_start(grep_t[:], grep_ext.ap().rearrange("l p h -> p l h"))
        brep_t = pers.tile([128, L, H], DT.float32, tag="brep")
        nc.sync.dma_start(brep_t[:], brep_ext.ap().rearrange("l p h -> p l h"))
        identbf_t = pers.tile([128, 128], DT.bfloat16, tag="identbf")
        nc.sync.dma_start(identbf_t[:], identbf_ext[:, :])
        identf_t = pers.tile([128, 128], DT.float32, tag="identf")
        nc.sync.dma_start(identf_t[:], identf_ext[:, :])
        hW1_t = pers.tile([H, 64], DT.float32, tag="hW1")
        nc.sync.dma_start(hW1_t[:], hW1_ext[:, :])
        hb1_t = pers.tile([64, 1], DT.float32, tag="hb1")
        nc.sync.dma_start(hb1_t[:], hb1_ext[:, :])
        hW2_t = pers.tile([64, 32], DT.float32, tag="hW2")
        nc.sync.dma_start(hW2_t[:], hW2_ext[:, :])
        hb2_t = pers.tile([32, 1], DT.float32, tag="hb2")
        nc.sync.dma_start(hb2_t[:], hb2_ext[:, :])
        hW3_t = pers.tile([32, 1], DT.float32, tag="hW3")
        nc.sync.dma_start(hW3_t[:], hW3_ext[:, :])
        hb3_t = pers.tile([1, 1], DT.float32, tag="hb3")
        nc.sync.dma_start(hb3_t[:], hb3_ext[:, :])

        hA = pers.tile([128, NW, H], DT.float32, tag="hA")
        hB = pers.tile([128, NW, H], DT.float32, tag="hB")
        hTfm = pers.tile([128, NW, H], DT.bfloat16, tag="hTfm")
        mnm = pers.tile([128, NW, H], DT.bfloat16, tag="mnm")

        for rep in range(nrep):
            # ---- encoder (feature-major) ----
            for t in range(NT512):
                z_ps = psB.tile([128, 512], DT.float32, tag="mm")
                nc.tensor.matmul(z_ps[:], lhsT=encW1_t[:],
                                 rhs=lcT_t[:, t * 512:(t + 1) * 512],
                                 start=True, stop=True)
                z_sb = sb.tile([128, 512], DT.float32, tag="zenc")
                nc.scalar.activation(z_sb[:], z_ps[:], AF.Relu, bias=encb1_t[:, 0:1])
                h0_ps = psC.tile([128, 512], DT.float32, tag="tr")
                nc.tensor.matmul(h0_ps[:], lhsT=encW2_t[:], rhs=z_sb[:],
                                 start=True, stop=True)
                hTv = hTfm[:].rearrange("p w h -> p (w h)")
                nc.scalar.activation(hTv[:, t * 512:(t + 1) * 512], h0_ps[:],
                                     AF.Identity, bias=encb2_t[:, 0:1])
            hTv = hTfm[:].rearrange("p w h -> p (w h)")
            nc.vector.memset(hTv[:, SHARD:SH], 0.0)

            h_in, h_out = hA, hB
            for l in range(L):
                # ---- m~ table path ----
                for t in range(NT512):
                    m_ps = psB.tile([128, 512], DT.float32, tag="mm")
                    nc.tensor.matmul(
                        m_ps[:], lhsT=convW_t[:, l, :],
                        rhs=hTfm[:].rearrange("p w h -> p (w h)")[:, t * 512:(t + 1) * 512],
                        start=True, stop=True)
                    m_fm = sb.tile([128, 512], DT.bfloat16, tag="mfm")
                    nc.vector.tensor_copy(m_fm[:], m_ps[:])
                    for j in range(4):
                        w_ = 4 * t + j
                        mT_ps = psC.tile([128, 128], DT.bfloat16, tag="tr")
                        nc.tensor.transpose(mT_ps[:], m_fm[:, j * 128:(j + 1) * 128],
                                            identbf_t[:])
                        nc.vector.tensor_scalar(
                            out=mnm[:, w_, :], in0=mT_ps[:],
                            scalar1=dis_t[:, w_:w_ + 1], scalar2=None,
                            op0=ALU.mult)
                nc.sync.dma_start(mloc_d, mnm[:].rearrange("p w h -> p (w h)"))
                nc.sync.dma_start(mloc_d[106:107, 48 * H:49 * H], convb_t[:, l, :])
                nc.gpsimd.collective_compute(
                    "AllGather", ALU.bypass,
                    ins=[mloc_d.opt()], outs=[table_d.opt()],
                    replica_groups=[list(range(P))],
                )

                # ---- edge aggregation ----
                for b in range(BANKS):
                    aggw = [psA.tile([128, 128], DT.float32, tag="aggw",
                                     name=f"aggw_{rep}_{l}_{b}_{_wi}")
                            for _wi in range(4)]
                    for h_sel in range(2):
                        ks = [k for k, s_ in enumerate(seq)
                              if s_[0] == b and s_[1] == h_sel]
                        for bi in range(0, len(ks), BATCH):
                            nck = min(BATCH, len(ks) - bi)
                            k0 = ks[bi]
                            msg = msgs.tile([128, BATCH, H], DT.bfloat16, tag="msg")
                            s_t = sstream.tile([128, BATCH, W], DT.bfloat16, tag="S")
                            nc.sync.dma_start(s_t[:, 0:nck, :], S_ext[:, k0:k0 + nck, :])
                            base = h_sel * HALF
                            nc.gpsimd.dma_gather(
                                msg[:, 0:nck, :], table_d[base:base + HALF, :],
                                idx_t[:, k0 * 8:(k0 + nck) * 8],
                                nck * 128, nck * 128, H,
                                single_packet=False,
                            )
                            for j in range(nck):
                                k = k0 + j
                                _, h_, wi_, st_ = seq[k]
                                nc.tensor.matmul(
                                    aggw[wi_][:], lhsT=s_t[:, j, :], rhs=msg[:, j, :],
                                    start=st_, stop=False, skip_group_check=True)
                    for wi in range(4):
                        w_ = 4 * b + wi
                        nc.tensor.matmul(aggw[wi][:], lhsT=identbf_t[:],
                                         rhs=mnm[:, w_, :], start=False, stop=True,
                                         skip_group_check=True)
                    agg = sb.tile([128, 4, 128], DT.float32, tag="aggsb")
                    for wi in range(4):
                        nc.vector.tensor_copy(agg[:, wi, :], aggw[wi][:])
                    # ---- LayerNorm ----
                    st1 = sb.tile([128, 4], DT.float32, tag="st1")
                    st2 = sb.tile([128, 4], DT.float32, tag="st2")
                    sq = sb.tile([128, 4, 128], DT.float32, tag="sq")
                    nc.vector.tensor_reduce(st1[:], agg[:], mybir.AxisListType.X, ALU.add)
                    nc.vector.tensor_tensor(sq[:], agg[:], agg[:], ALU.mult)
                    nc.vector.tensor_reduce(st2[:], sq[:], mybir.AxisListType.X, ALU.add)
                    mu = sb.tile([128, 4], DT.float32, tag="mu")
                    nc.vector.tensor_scalar(out=mu[:], in0=st1[:], scalar1=1.0 / H,
                                            scalar2=None, op0=ALU.mult)
                    var = sb.tile([128, 4], DT.float32, tag="var")
                    nc.vector.tensor_scalar(out=var[:], in0=st2[:], scalar1=1.0 / H,
                                            scalar2=None, op0=ALU.mult)
                    mu2 = sb.tile([128, 4], DT.float32, tag="mu2")
                    nc.vector.tensor_tensor(mu2[:], mu[:], mu[:], ALU.mult)
                    nc.vector.tensor_tensor(var[:], var[:], mu2[:], ALU.subtract)
                    nc.vector.tensor_tensor(var[:], var[:], eps_t[:, 4 * b:4 * b + 4],
                                            ALU.add)
                    std = sb.tile([128, 4], DT.float32, tag="std")
                    nc.scalar.sqrt(std[:], var[:])
                    rstd = sb.tile([128, 4], DT.float32, tag="rstd")
                    nc.vector.reciprocal(rstd[:], std[:])
                    tmp = sb.tile([128, 4, 128], DT.float32, tag="tmp")
                    nc.vector.tensor_tensor(
                        tmp[:], agg[:], mu[:, :, None].broadcast_to([128, 4, 128]),
                        ALU.subtract)
                    nc.vector.tensor_tensor(
                        tmp[:], tmp[:], rstd[:, :, None].broadcast_to([128, 4, 128]),
                        ALU.mult)
                    nc.vector.tensor_tensor(
                        tmp[:], tmp[:],
                        grep_t[:, l, :][:, None, :].broadcast_to([128, 4, 128]),
                        ALU.mult)
                    nc.vector.tensor_tensor(
                        tmp[:], tmp[:],
                        brep_t[:, l, :][:, None, :].broadcast_to([128, 4, 128]),
                        ALU.add)
                    if l > 0:
                        nc.vector.tensor_scalar(out=tmp[:], in0=tmp[:], scalar1=0.0,
                                                scalar2=None, op0=ALU.max)
                        nc.vector.tensor_tensor(h_out[:, 4 * b:4 * b + 4, :], tmp[:],
                                                h_in[:, 4 * b:4 * b + 4, :], ALU.add)
                    else:
                        nc.vector.tensor_scalar(out=h_out[:, 4 * b:4 * b + 4, :],
                                                in0=tmp[:], scalar1=0.0,
                                                scalar2=None, op0=ALU.max)

                if l < L - 1:
                    for w_ in range(NW):
                        hT_ps = psC.tile([128, 128], DT.float32, tag="tr")
                        nc.tensor.transpose(hT_ps[:], h_out[:, w_, :], identf_t[:])
                        nc.vector.tensor_copy(hTfm[:, w_, :], hT_ps[:])
                h_in, h_out = h_out, h_in

            h_fin = h_in
            # ---- pooling ----
            pool_ps = psB.tile([G, H], DT.float32, tag="mm")
            for w_ in range(NW):
                nc.tensor.matmul(pool_ps[:], lhsT=B_t[:, w_, :], rhs=h_fin[:, w_, :],
                                 start=(w_ == 0), stop=(w_ == NW - 1))
            pool_sb = sb.tile([G, H], DT.float32, tag="pool")
            nc.vector.tensor_copy(pool_sb[:], pool_ps[:])
            nc.sync.dma_start(pool_loc_d, pool_sb[:])
            nc.gpsimd.collective_compute(
                "AllReduce", ALU.add,
                ins=[pool_loc_d.opt()], outs=[pool_full_d.opt()],
                replica_groups=[list(range(P))],
            )
            poolf = sb.tile([G, H], DT.float32, tag="poolf")
            nc.sync.dma_start(poolf[:], pool_full_d)
            # ---- head ----
            poolT_ps = psC.tile([128, G], DT.float32, tag="tr")
            nc.tensor.transpose(poolT_ps[:], poolf[:], identf_t[:G, :G])
            poolT = sb.tile([128, G], DT.float32, tag="poolT")
            nc.vector.tensor_copy(poolT[:], poolT_ps[:])
            z1_ps = psB.tile([64, G], DT.float32, tag="mm")
            nc.tensor.matmul(z1_ps[:], lhsT=hW1_t[:], rhs=poolT[:], start=True, stop=True)
            z1 = sb.tile([64, G], DT.float32, tag="z1")
            nc.scalar.activation(z1[:], z1_ps[:], AF.Relu, bias=hb1_t[:, 0:1])
            z2_ps = psB.tile([32, G], DT.float32, tag="mm")
            nc.tensor.matmul(z2_ps[:], lhsT=hW2_t[:], rhs=z1[:], start=True, stop=True)
            z2 = sb.tile([32, G], DT.float32, tag="z2")
            nc.scalar.activation(z2[:], z2_ps[:], AF.Relu, bias=hb2_t[:, 0:1])
            z3_ps = psB.tile([1, G], DT.float32, tag="mm")
            nc.tensor.matmul(z3_ps[:], lhsT=hW3_t[:], rhs=z2[:], start=True, stop=True)
            # softplus(x) = relu(x) + ln(1 + exp(-|x|))
            x_sb = sb.tile([1, G], DT.float32, tag="oi")
            nc.scalar.activation(x_sb[:], z3_ps[:], AF.Identity, bias=hb3_t[:, 0:1])
            ax = sb.tile([1, G], DT.float32, tag="ax")
            nc.scalar.activation(ax[:], x_sb[:], AF.Abs)
            ex = sb.tile([1, G], DT.float32, tag="ex")
            nc.scalar.activation(ex[:], ax[:], AF.Exp, scale=-1.0)
            lx = sb.tile([1, G], DT.float32, tag="lx")
            nc.scalar.activation(lx[:], ex[:], AF.Ln, bias=1.0)
            rx = sb.tile([1, G], DT.float32, tag="rx")
            nc.scalar.activation(rx[:], x_sb[:], AF.Relu)
            oi = sb.tile([1, G], DT.float32, tag="oi2")
            nc.vector.tensor_tensor(oi[:], lx[:], rx[:], ALU.add)
            nc.sync.dma_start(out_ext.ap().rearrange("g x -> x g"), oi[:])

    nc.compile()
    return nc


class SpmdRunner:
    def __init__(self, nc, n_cores=P):
        import jax
        import concourse.mybir as mybir
        from concourse import bass2jax
        from jax.sharding import Mesh, PartitionSpec
        from jax.experimental.shard_map import shard_map

        bass2jax.install_neuronx_cc_hook()
        self.n_cores = n_cores
        in_names, out_names, out_avals, zero_outs = [], [], [], []
        partition_name = nc.partition_id_tensor.name if nc.partition_id_tensor else None
        for alloc in nc.m.functions[0].allocations:
            if not isinstance(alloc, mybir.MemoryLocationSet):
                continue
            name = alloc.memorylocations[0].name
            if alloc.kind == "ExternalInput":
                if name != partition_name:
                    in_names.append(name)
            elif alloc.kind == "ExternalOutput":
                out_names.append(name)
                shape = tuple(alloc.tensor_shape)
                dtype = mybir.dt.np(alloc.dtype)
                out_avals.append(jax.core.ShapedArray(shape, dtype))
                zero_outs.append(np.zeros(shape, dtype))
        self.in_names, self.out_names = in_names, out_names
        self.out_avals, self.zero_outs = out_avals, zero_outs
        n_params, n_outs = len(in_names), len(out_names)
        all_in_names = list(in_names) + list(out_names)
        if partition_name is not None:
            all_in_names.append(partition_name)

        def _body(*args):
            operands = list(args)
            if partition_name is not None:
                operands.append(bass2jax.partition_id_tensor())
            outs = bass2jax._bass_exec_p.bind(
                *operands,
                out_avals=tuple(out_avals),
                in_names=tuple(all_in_names),
                out_names=tuple(out_names),
                lowering_input_output_aliases=(),
                sim_require_finite=True,
                sim_require_nnan=True,
                nc=nc,
            )
            return tuple(outs)

        devices = jax.devices()[:n_cores]
        mesh = Mesh(np.asarray(devices), ("core",))
        in_specs = (PartitionSpec("core"),) * (n_params + n_outs)
        out_specs = (PartitionSpec("core"),) * n_outs
        self.sharded = jax.jit(
            shard_map(_body, mesh=mesh, in_specs=in_specs,
                      out_specs=out_specs, check_rep=False),
            keep_unused=True,
        )
        self._jax = jax

    def prepare(self, in_maps):
        jax = self._jax
        n = self.n_cores
        concat_in = [
            np.concatenate([np.asarray(in_maps[c][name]) for c in range(n)], axis=0)
            for name in self.in_names
        ]
        concat_zeros = [
            np.zeros((n * z.shape[0], *z.shape[1:]), z.dtype) for z in self.zero_outs
        ]
        self.args = [jax.device_put(a) for a in concat_in + concat_zeros]

    def run(self):
        jax = self._jax
        outs = self.sharded(*self.args)
        jax.block_until_ready(outs)
        return [
            {
                name: np.asarray(outs[i]).reshape(self.n_cores, *self.out_avals[i].shape)[c]
                for i, name in enumerate(self.out_names)
            }
            for c in range(self.n_cores)
        ]

    def time_it(self, iters=12, warmup=2):
        import time
        jax = self._jax
        for _ in range(warmup):
            jax.block_until_ready(self.sharded(*self.args))
        times = []
        for _ in range(iters):
            t0 = time.perf_counter()
            jax.block_until_ready(self.sharded(*self.args))
            times.append(time.perf_counter() - t0)
        return min(times), float(np.median(times))


_CACHE = {}


def _get_runner(inputs, nrep=1):
    import zlib
    struct, per_core, consts = build_host(inputs)
    _nonce = (zlib.crc32((repr(struct["seq"]) + "v5" + str(nrep)).encode()) % 997) + 2
    key = (struct["NCH"], nrep)
    if key not in _CACHE:
        nc = build_nc(struct, nrep=nrep)
        _CACHE[key] = SpmdRunner(nc, P)
    runner = _CACHE[key]
    in_maps = []
    for c in range(P):
        m = dict(consts)
        m.update({
            "nonce": np.zeros((1, _nonce), np.float32),
            "idx": per_core[c]["idx"], "S": per_core[c]["S"],
            "lcT": per_core[c]["lcT"], "dis_tile": per_core[c]["dis_tile"],
            "eps_tile": per_core[c]["eps_tile"], "B_tile": per_core[c]["B_tile"],
        })
        in_maps.append(m)
    runner.prepare(in_maps)
    return runner


def kernel(**inputs):
    runner = _get_runner(inputs, nrep=1)
    outs = runner.run()
    return outs[0]["out"].astype(np.float32)


if __name__ == "__main__":
    d = np.load("/root/problem/dev/ref_inputs.npz")
    inputs = {k: d[k] for k in d.files}
    out = kernel(**inputs)
    print(out[:4].ravel())



# revision 20
# speedup vs baseline: 1.7884x; 1.7884x over previous
"""ALCDEF Temporal GNN (gnn_message_passing) on 8 TRN2 NeuronCores.

Self-contained: takes FULL unsharded inputs, returns FULL [64,1] output.

Strategy (all shapes hardcoded for N=50000, E=800000, F_IN=16, H=128, L=3, G=64):
- Nodes dst-sharded across 8 cores (6250 each, padded to 6656 local slots;
  local node l = w*128 + p for window w, partition p).
- Per layer each core computes its shard of the message table
  m~[n] = dis_n * (h @ W_l) (node-major bf16 rows, row r = c*6656 + p*52 + w)
  and AllGathers it so every core holds the full table in DRAM.
- Edge aggregation: SWDGE dma_gather pulls 1024 message rows per call
  (edge-major msgT [128 edges, 128 feat]); one-hot S matrices (streamed
  bf16, built on host) scatter-add them into per-window PSUM banks via
  TensorE matmuls; self-loops are injected with an identity matmul from
  the local node-major m~ copy; the GCN bias rides a dedicated table row
  with S value 1/dis_d.
- LayerNorm is computed node-major per PSUM bank with an exact per-node
  eps correction (eps_d = eps*deg_d) that makes the dis_d folding exact.
- Mean-pool via per-window matmuls with a host-built B matrix, AllReduce,
  then the small head MLP (softplus composed from Relu/Exp/Ln) on every
  core; core 0's output is returned.
"""
import sys
sys.path.insert(0, "/opt/trn_rl_repo")
import numpy as np
import ml_dtypes
from contextlib import ExitStack

bf16 = ml_dtypes.bfloat16
f32 = np.float32

N, E, F_IN, H, L, G = 50000, 800000, 16, 128, 3, 64
LN_EPS = 1e-5
P = 8
SH = 6656
W = 128
NW = SH // W          # 52
BANKS = NW // 4       # 13
SHARD = 6250
HALF = 4 * SH         # 26624 table rows per half
BATCH = 8             # chunks per gather call (1024 idxs)
NQ = 2                # SWDGE queues (desc-gen pipelines across Q7 core pairs)
NT512 = SH // 512     # 13


def _row_of_local(l):
    w, p = l // W, l % W
    return p * NW + w


def build_host(inputs):
    edge_index = np.asarray(inputs["edge_index"])
    batch = np.asarray(inputs["batch"]).astype(np.int64)
    src_g = edge_index[0].astype(np.int64)
    dst_g = edge_index[1].astype(np.int64)

    deg = np.bincount(dst_g, minlength=N).astype(np.float64) + 1.0
    dis = (1.0 / np.sqrt(deg)).astype(np.float64)

    own = np.minimum(src_g // SHARD, P - 1)
    src_loc = src_g - own * SHARD
    src_row = own * SH + (src_loc % W) * NW + (src_loc // W)

    own_d = np.minimum(dst_g // SHARD, P - 1)
    d_loc = dst_g - own_d * SHARD
    win = d_loc // W
    half = (src_row >= HALF).astype(np.int64)

    BROW = _row_of_local(6250)
    ZROW = _row_of_local(6251)

    # one gather slot per distinct (window, half, src); S row holds all its
    # dst columns (values = edge multiplicity)
    slot_cnt = np.zeros((P, NW, 2), dtype=np.int64)
    for c in range(P):
        m = own_d == c
        for h in range(2):
            mm_ = m & (half == h)
            pairs = win[mm_] * (HALF + 1) + src_row[mm_] - h * HALF
            slot_cnt[c, :, h] = np.bincount(
                np.unique(pairs) // (HALF + 1), minlength=NW)
    slot_cnt[:, :, 0] += 1  # bias slot in half A
    chunks_max = np.ceil(slot_cnt / 128).astype(np.int64).max(axis=0)

    seq = []  # (bank, half, wi, start)
    for b in range(BANKS):
        for h in range(2):
            for wi in range(4):
                for k in range(chunks_max[4 * b + wi, h]):
                    seq.append((b, h, wi, k == 0 and h == 0))
    NCH = len(seq)
    struct = {"seq": seq, "NCH": NCH}

    per_core = []
    lc = np.asarray(inputs["lightcurve"], f32)
    counts = np.bincount(batch, minlength=G).astype(np.float64)
    for c in range(P):
        m = np.flatnonzero(own_d == c)
        e_half = half[m]
        e_win = win[m]
        e_row = src_row[m] - e_half * HALF
        e_col = d_loc[m] % W

        idx_blob = np.zeros((NCH, 128), dtype=np.int16)
        S = np.zeros((NCH, 128, W), dtype=f32)

        pos = {}
        for k, (b, h, wi, st) in enumerate(seq):
            if wi >= 0:
                pos.setdefault((4 * b + wi, h), []).append(k)
        for (w_, h), ks in pos.items():
            sel = np.flatnonzero((e_win == w_) & (e_half == h))
            rows_e = e_row[sel]
            cols_e = e_col[sel]
            uniq, inv = np.unique(rows_e, return_inverse=True)
            nslots = len(uniq)
            cap = len(ks) * 128
            bias_slot = nslots if h == 0 else -1
            total = nslots + (1 if h == 0 else 0)
            assert total <= cap, (w_, h, total, cap)
            slot_rows = np.full(cap, ZROW, dtype=np.int16)
            slot_rows[:nslots] = uniq.astype(np.int16)
            if h == 0:
                slot_rows[bias_slot] = BROW
            for j, k in enumerate(ks):
                idx_blob[k] = slot_rows[j * 128:(j + 1) * 128]
            kk = np.array(ks)[inv // 128]
            np.add.at(S, (kk, inv % 128, cols_e), 1.0)
            if h == 0:
                kb = ks[bias_slot // 128]
                node0 = w_ * W
                g0 = c * SHARD
                nreal = min(W, SHARD - node0)
                if nreal > 0:
                    S[kb, bias_slot % 128, :nreal] = \
                        1.0 / dis[g0 + node0:g0 + node0 + nreal]

        wrapped = idx_blob.reshape(NCH, 8, 16).transpose(2, 0, 1).reshape(16, NCH * 8)
        wrapped = np.tile(wrapped, (8, 1)).astype(np.int16)

        g0 = c * SHARD
        dis_loc = np.zeros(SH, f32)
        dis_loc[:SHARD] = dis[g0:g0 + SHARD]
        eps_loc = np.full(SH, LN_EPS, f32)
        eps_loc[:SHARD] = (LN_EPS * deg[g0:g0 + SHARD]).astype(f32)
        dis_tile = dis_loc.reshape(NW, W).T.copy()
        eps_tile = eps_loc.reshape(NW, W).T.copy()

        Bp = np.zeros((SH, G), f32)
        bb = batch[g0:g0 + SHARD]
        Bp[np.arange(SHARD), bb] = (1.0 / np.maximum(counts[bb], 1.0)).astype(f32)
        B_tile = Bp.reshape(NW, W, G).transpose(1, 0, 2).copy()

        lcT = np.zeros((F_IN, SH), f32)
        lcT[:, :SHARD] = lc[g0:g0 + SHARD].T

        per_core.append({
            "idx": np.ascontiguousarray(wrapped),
            "S": np.ascontiguousarray(S.transpose(1, 0, 2)).astype(bf16),
            "dis_tile": dis_tile, "eps_tile": eps_tile,
            "B_tile": np.ascontiguousarray(B_tile),
            "lcT": lcT,
        })

    consts = {
        "enc_W1": np.asarray(inputs["enc_W1"], f32),
        "enc_b1": np.asarray(inputs["enc_b1"], f32).reshape(H, 1),
        "enc_W2": np.asarray(inputs["enc_W2"], f32),
        "enc_b2": np.asarray(inputs["enc_b2"], f32).reshape(H, 1),
        "convW": np.asarray(inputs["conv_W"], f32).astype(bf16),
        "conv_b_bf": np.asarray(inputs["conv_b"], f32).astype(bf16).reshape(L, 1, H),
        "g_rep": np.tile(np.asarray(inputs["ln_g"], f32)[:, None, :], (1, 128, 1)),
        "b_rep": np.tile(np.asarray(inputs["ln_b"], f32)[:, None, :], (1, 128, 1)),
        "ident_bf": np.eye(128, dtype=f32).astype(bf16),
        "ident_f32": np.eye(128, dtype=f32),
        "hW1": np.asarray(inputs["h_W1"], f32),
        "hb1": np.asarray(inputs["h_b1"], f32).reshape(64, 1),
        "hW2": np.asarray(inputs["h_W2"], f32),
        "hb2": np.asarray(inputs["h_b2"], f32).reshape(32, 1),
        "hW3": np.asarray(inputs["h_W3"], f32),
        "hb3": np.asarray(inputs["h_b3"], f32).reshape(1, 1),
    }
    return struct, per_core, consts


def build_nc(struct, nrep=1):
    import zlib
    nonce = (zlib.crc32((repr(struct["seq"]) + "v7" + str(nrep)).encode()) % 997) + 2
    import concourse.bass as bass
    import concourse.bacc as bacc
    import concourse.mybir as mybir
    import concourse.tile as tile
    from concourse.library_config import mlp as mlp_lib

    seq = struct["seq"]
    NCH = struct["NCH"]
    AF = mybir.ActivationFunctionType
    ALU = mybir.AluOpType
    DT = mybir.dt

    nc = bacc.Bacc("TRN2", debug=False, num_devices=P, num_swdge_queues=NQ)
    dp = nc.declare_dram_parameter
    idx_ext = dp("idx", [128, NCH * 8], DT.int16, isOutput=False)
    S_ext = dp("S", [128, NCH, W], DT.bfloat16, isOutput=False)
    lcT_ext = dp("lcT", [F_IN, SH], DT.float32, isOutput=False)
    dis_ext = dp("dis_tile", [128, NW], DT.float32, isOutput=False)
    eps_ext = dp("eps_tile", [128, NW], DT.float32, isOutput=False)
    B_ext = dp("B_tile", [128, NW, G], DT.float32, isOutput=False)
    encW1_ext = dp("enc_W1", [F_IN, H], DT.float32, isOutput=False)
    encb1_ext = dp("enc_b1", [H, 1], DT.float32, isOutput=False)
    encW2_ext = dp("enc_W2", [H, H], DT.float32, isOutput=False)
    encb2_ext = dp("enc_b2", [H, 1], DT.float32, isOutput=False)
    convW_ext = dp("convW", [L, H, H], DT.bfloat16, isOutput=False)
    convb_ext = dp("conv_b_bf", [L, 1, H], DT.bfloat16, isOutput=False)
    grep_ext = dp("g_rep", [L, 128, H], DT.float32, isOutput=False)
    brep_ext = dp("b_rep", [L, 128, H], DT.float32, isOutput=False)
    identbf_ext = dp("ident_bf", [128, 128], DT.bfloat16, isOutput=False)
    identf_ext = dp("ident_f32", [128, 128], DT.float32, isOutput=False)
    hW1_ext = dp("hW1", [H, 64], DT.float32, isOutput=False)
    hb1_ext = dp("hb1", [64, 1], DT.float32, isOutput=False)
    hW2_ext = dp("hW2", [64, 32], DT.float32, isOutput=False)
    hb2_ext = dp("hb2", [32, 1], DT.float32, isOutput=False)
    hW3_ext = dp("hW3", [32, 1], DT.float32, isOutput=False)
    hb3_ext = dp("hb3", [1, 1], DT.float32, isOutput=False)
    out_ext = dp("out", [G, 1], DT.float32, isOutput=True)
    dp("nonce", [1, nonce], DT.float32, isOutput=False)

    mloc_th = nc.dram_tensor("mloc_i", [128, NW * H], DT.bfloat16)
    table_th = nc.dram_tensor("table_i", [P * SH, H], DT.bfloat16, addr_space="Shared")
    pool_loc_th = nc.dram_tensor("pool_loc_i", [G, H], DT.float32)
    pool_full_th = nc.dram_tensor("pool_full_i", [G, H], DT.float32, addr_space="Shared")

    with tile.TileContext(nc) as tc, ExitStack() as ctx:
        mloc_d = mloc_th.ap()
        table_d = table_th.ap()
        pool_loc_d = pool_loc_th.ap()
        pool_full_d = pool_full_th.ap()
        nc.gpsimd.load_library(mlp_lib)
        pers = ctx.enter_context(tc.tile_pool(name="pers", bufs=1))
        sb = ctx.enter_context(tc.tile_pool(name="sb", bufs=3))
        sstream = ctx.enter_context(tc.tile_pool(name="sstream", bufs=7))
        msgs = ctx.enter_context(tc.tile_pool(name="msgs", bufs=9))
        psA = ctx.enter_context(tc.tile_pool(name="psA", bufs=4, space="PSUM"))
        psB = ctx.enter_context(tc.tile_pool(name="psB", bufs=2, space="PSUM"))
        psC = ctx.enter_context(tc.tile_pool(name="psC", bufs=2, space="PSUM"))

        idx_t = pers.tile([128, NCH * 8], DT.int16, tag="idx")
        nc.sync.dma_start(idx_t[:], idx_ext[:, :])
        dis_t = pers.tile([128, NW], DT.float32, tag="dis")
        nc.sync.dma_start(dis_t[:], dis_ext[:, :])
        eps_t = pers.tile([128, NW], DT.float32, tag="eps")
        nc.sync.dma_start(eps_t[:], eps_ext[:, :])
        B_t = pers.tile([128, NW, G], DT.float32, tag="B")
        nc.sync.dma_start(B_t[:], B_ext[:, :, :])
        lcT_t = pers.tile([F_IN, SH], DT.float32, tag="lcT")
        nc.sync.dma_start(lcT_t[:], lcT_ext[:, :])
        encW1_t = pers.tile([F_IN, H], DT.float32, tag="encW1")
        nc.sync.dma_start(encW1_t[:], encW1_ext[:, :])
        encb1_t = pers.tile([H, 1], DT.float32, tag="encb1")
        nc.sync.dma_start(encb1_t[:], encb1_ext[:, :])
        encW2_t = pers.tile([H, H], DT.float32, tag="encW2")
        nc.sync.dma_start(encW2_t[:], encW2_ext[:, :])
        encb2_t = pers.tile([H, 1], DT.float32, tag="encb2")
        nc.sync.dma_start(encb2_t[:], encb2_ext[:, :])
        convW_t = pers.tile([H, L, H], DT.bfloat16, tag="convW")
        nc.sync.dma_start(convW_t[:], convW_ext.ap().rearrange("l a b -> a l b"))
        convb_t = pers.tile([1, L, H], DT.bfloat16, tag="convb")
        nc.sync.dma_start(convb_t[:], convb_ext.ap().rearrange("l a b -> a l b"))
        grep_t = pers.tile([128, L, H], DT.float32, tag="grep")
        nc.sync.dma_start(grep_t[:], grep_ext.ap().rearrange("l p h -> p l h"))
        brep_t = pers.tile([128, L, H], DT.float32, tag="brep")
        nc.sync.dma_start(brep_t[:], brep_ext.ap().rearrange("l p h -> p l h"))
        identbf_t = pers.tile([128, 128], DT.bfloat16, tag="identbf")
        nc.sync.dma_start(identbf_t[:], identbf_ext[:, :])
        identf_t = pers.tile([128, 128], DT.float32, tag="identf")
        nc.sync.dma_start(identf_t[:], identf_ext[:, :])
        hW1_t = pers.tile([H, 64], DT.float32, tag="hW1")
        nc.sync.dma_start(hW1_t[:], hW1_ext[:, :])
        hb1_t = pers.tile([64, 1], DT.float32, tag="hb1")
        nc.sync.dma_start(hb1_t[:], hb1_ext[:, :])
        hW2_t = pers.tile([64, 32], DT.float32, tag="hW2")
        nc.sync.dma_start(hW2_t[:], hW2_ext[:, :])
        hb2_t = pers.tile([32, 1], DT.float32, tag="hb2")
        nc.sync.dma_start(hb2_t[:], hb2_ext[:, :])
        hW3_t = pers.tile([32, 1], DT.float32, tag="hW3")
        nc.sync.dma_start(hW3_t[:], hW3_ext[:, :])
        hb3_t = pers.tile([1, 1], DT.float32, tag="hb3")
        nc.sync.dma_start(hb3_t[:], hb3_ext[:, :])

        hA = pers.tile([128, NW, H], DT.float32, tag="hA")
        hB = pers.tile([128, NW, H], DT.float32, tag="hB")
        hTfm = pers.tile([128, NW, H], DT.bfloat16, tag="hTfm")
        mnm = pers.tile([128, NW, H], DT.bfloat16, tag="mnm")

        for rep in range(nrep):
            # ---- encoder (feature-major) ----
            for t in range(NT512):
                z_ps = psB.tile([128, 512], DT.float32, tag="mm")
                nc.tensor.matmul(z_ps[:], lhsT=encW1_t[:],
                                 rhs=lcT_t[:, t * 512:(t + 1) * 512],
                                 start=True, stop=True)
                z_sb = sb.tile([128, 512], DT.float32, tag="zenc")
                nc.scalar.activation(z_sb[:], z_ps[:], AF.Relu, bias=encb1_t[:, 0:1])
                h0_ps = psC.tile([128, 512], DT.float32, tag="tr")
                nc.tensor.matmul(h0_ps[:], lhsT=encW2_t[:], rhs=z_sb[:],
                                 start=True, stop=True)
                hTv = hTfm[:].rearrange("p w h -> p (w h)")
                nc.scalar.activation(hTv[:, t * 512:(t + 1) * 512], h0_ps[:],
                                     AF.Identity, bias=encb2_t[:, 0:1])
            hTv = hTfm[:].rearrange("p w h -> p (w h)")
            nc.vector.memset(hTv[:, SHARD:SH], 0.0)

            h_in, h_out = hA, hB
            for l in range(L):
                # ---- m~ table path ----
                for t in range(NT512):
                    m_ps = psB.tile([128, 512], DT.float32, tag="mm")
                    nc.tensor.matmul(
                        m_ps[:], lhsT=convW_t[:, l, :],
                        rhs=hTfm[:].rearrange("p w h -> p (w h)")[:, t * 512:(t + 1) * 512],
                        start=True, stop=True)
                    m_fm = sb.tile([128, 512], DT.bfloat16, tag="mfm")
                    nc.vector.tensor_copy(m_fm[:], m_ps[:])
                    for j in range(4):
                        w_ = 4 * t + j
                        mT_ps = psC.tile([128, 128], DT.bfloat16, tag="tr")
                        nc.tensor.transpose(mT_ps[:], m_fm[:, j * 128:(j + 1) * 128],
                                            identbf_t[:])
                        nc.vector.tensor_scalar(
                            out=mnm[:, w_, :], in0=mT_ps[:],
                            scalar1=dis_t[:, w_:w_ + 1], scalar2=None,
                            op0=ALU.mult)
                nc.sync.dma_start(mloc_d, mnm[:].rearrange("p w h -> p (w h)"))
                nc.sync.dma_start(mloc_d[106:107, 48 * H:49 * H], convb_t[:, l, :])
                nc.gpsimd.collective_compute(
                    "AllGather", ALU.bypass,
                    ins=[mloc_d.opt()], outs=[table_d.opt()],
                    replica_groups=[list(range(P))],
                )

                # ---- edge aggregation ----
                gq = 0
                for b in range(BANKS):
                    aggw = [psA.tile([128, 128], DT.float32, tag="aggw",
                                     name=f"aggw_{rep}_{l}_{b}_{_wi}")
                            for _wi in range(4)]
                    for h_sel in range(2):
                        ks = [k for k, s_ in enumerate(seq)
                              if s_[0] == b and s_[1] == h_sel]
                        for bi in range(0, len(ks), BATCH):
                            nck = min(BATCH, len(ks) - bi)
                            k0 = ks[bi]
                            msg = msgs.tile([128, BATCH, H], DT.bfloat16, tag="msg")
                            s_t = sstream.tile([128, BATCH, W], DT.bfloat16, tag="S")
                            nc.sync.dma_start(s_t[:, 0:nck, :], S_ext[:, k0:k0 + nck, :])
                            base = h_sel * HALF
                            nc.gpsimd.dma_gather(
                                msg[:, 0:nck, :], table_d[base:base + HALF, :],
                                idx_t[:, k0 * 8:(k0 + nck) * 8],
                                nck * 128, nck * 128, H,
                                single_packet=False,
                                queue_num=gq % NQ,
                            )
                            gq += 1
                            for j in range(nck):
                                k = k0 + j
                                _, h_, wi_, st_ = seq[k]
                                nc.tensor.matmul(
                                    aggw[wi_][:], lhsT=s_t[:, j, :], rhs=msg[:, j, :],
                                    start=st_, stop=False, skip_group_check=True)
                    for wi in range(4):
                        w_ = 4 * b + wi
                        nc.tensor.matmul(aggw[wi][:], lhsT=identbf_t[:],
                                         rhs=mnm[:, w_, :], start=False, stop=True,
                                         skip_group_check=True)
                    agg = sb.tile([128, 4, 128], DT.float32, tag="aggsb")
                    for wi in range(4):
                        nc.vector.tensor_copy(agg[:, wi, :], aggw[wi][:])
                    # ---- LayerNorm ----
                    st1 = sb.tile([128, 4], DT.float32, tag="st1")
                    st2 = sb.tile([128, 4], DT.float32, tag="st2")
                    sq = sb.tile([128, 4, 128], DT.float32, tag="sq")
                    nc.vector.tensor_reduce(st1[:], agg[:], mybir.AxisListType.X, ALU.add)
                    nc.vector.tensor_tensor(sq[:], agg[:], agg[:], ALU.mult)
                    nc.vector.tensor_reduce(st2[:], sq[:], mybir.AxisListType.X, ALU.add)
                    mu = sb.tile([128, 4], DT.float32, tag="mu")
                    nc.vector.tensor_scalar(out=mu[:], in0=st1[:], scalar1=1.0 / H,
                                            scalar2=None, op0=ALU.mult)
                    var = sb.tile([128, 4], DT.float32, tag="var")
                    nc.vector.tensor_scalar(out=var[:], in0=st2[:], scalar1=1.0 / H,
                                            scalar2=None, op0=ALU.mult)
                    mu2 = sb.tile([128, 4], DT.float32, tag="mu2")
                    nc.vector.tensor_tensor(mu2[:], mu[:], mu[:], ALU.mult)
                    nc.vector.tensor_tensor(var[:], var[:], mu2[:], ALU.subtract)
                    nc.vector.tensor_tensor(var[:], var[:], eps_t[:, 4 * b:4 * b + 4],
                                            ALU.add)
                    std = sb.tile([128, 4], DT.float32, tag="std")
                    nc.scalar.sqrt(std[:], var[:])
                    rstd = sb.tile([128, 4], DT.float32, tag="rstd")
                    nc.vector.reciprocal(rstd[:], std[:])
                    tmp = sb.tile([128, 4, 128], DT.float32, tag="tmp")
                    nc.vector.tensor_tensor(
                        tmp[:], agg[:], mu[:, :, None].broadcast_to([128, 4, 128]),
                        ALU.subtract)
                    nc.vector.tensor_tensor(
                        tmp[:], tmp[:], rstd[:, :, None].broadcast_to([128, 4, 128]),
                        ALU.mult)
                    nc.vector.tensor_tensor(
                        tmp[:], tmp[:],
                        grep_t[:, l, :][:, None, :].broadcast_to([128, 4, 128]),
                        ALU.mult)
                    nc.vector.tensor_tensor(
                        tmp[:], tmp[:],
                        brep_t[:, l, :][:, None, :].broadcast_to([128, 4, 128]),
                        ALU.add)
                    if l > 0:
                        nc.vector.tensor_scalar(out=tmp[:], in0=tmp[:], scalar1=0.0,
                                                scalar2=None, op0=ALU.max)
                        nc.vector.tensor_tensor(h_out[:, 4 * b:4 * b + 4, :], tmp[:],
                                                h_in[:, 4 * b:4 * b + 4, :], ALU.add)
                    else:
                        nc.vector.tensor_scalar(out=h_out[:, 4 * b:4 * b + 4, :],
                                                in0=tmp[:], scalar1=0.0,
                                                scalar2=None, op0=ALU.max)

                if l < L - 1:
                    for w_ in range(NW):
                        hT_ps = psC.tile([128, 128], DT.float32, tag="tr")
                        nc.tensor.transpose(hT_ps[:], h_out[:, w_, :], identf_t[:])
                        nc.vector.tensor_copy(hTfm[:, w_, :], hT_ps[:])
                h_in, h_out = h_out, h_in

            h_fin = h_in
            # ---- pooling ----
            pool_ps = psB.tile([G, H], DT.float32, tag="mm")
            for w_ in range(NW):
                nc.tensor.matmul(pool_ps[:], lhsT=B_t[:, w_, :], rhs=h_fin[:, w_, :],
                                 start=(w_ == 0), stop=(w_ == NW - 1))
            pool_sb = sb.tile([G, H], DT.float32, tag="pool")
            nc.vector.tensor_copy(pool_sb[:], pool_ps[:])
            nc.sync.dma_start(pool_loc_d, pool_sb[:])
            nc.gpsimd.collective_compute(
                "AllReduce", ALU.add,
                ins=[pool_loc_d.opt()], outs=[pool_full_d.opt()],
                replica_groups=[list(range(P))],
            )
            poolf = sb.tile([G, H], DT.float32, tag="poolf")
            nc.sync.dma_start(poolf[:], pool_full_d)
            # ---- head ----
            poolT_ps = psC.tile([128, G], DT.float32, tag="tr")
            nc.tensor.transpose(poolT_ps[:], poolf[:], identf_t[:G, :G])
            poolT = sb.tile([128, G], DT.float32, tag="poolT")
            nc.vector.tensor_copy(poolT[:], poolT_ps[:])
            z1_ps = psB.tile([64, G], DT.float32, tag="mm")
            nc.tensor.matmul(z1_ps[:], lhsT=hW1_t[:], rhs=poolT[:], start=True, stop=True)
            z1 = sb.tile([64, G], DT.float32, tag="z1")
            nc.scalar.activation(z1[:], z1_ps[:], AF.Relu, bias=hb1_t[:, 0:1])
            z2_ps = psB.tile([32, G], DT.float32, tag="mm")
            nc.tensor.matmul(z2_ps[:], lhsT=hW2_t[:], rhs=z1[:], start=True, stop=True)
            z2 = sb.tile([32, G], DT.float32, tag="z2")
            nc.scalar.activation(z2[:], z2_ps[:], AF.Relu, bias=hb2_t[:, 0:1])
            z3_ps = psB.tile([1, G], DT.float32, tag="mm")
            nc.tensor.matmul(z3_ps[:], lhsT=hW3_t[:], rhs=z2[:], start=True, stop=True)
            # softplus(x) = relu(x) + ln(1 + exp(-|x|))
            x_sb = sb.tile([1, G], DT.float32, tag="oi")
            nc.scalar.activation(x_sb[:], z3_ps[:], AF.Identity, bias=hb3_t[:, 0:1])
            ax = sb.tile([1, G], DT.float32, tag="ax")
            nc.scalar.activation(ax[:], x_sb[:], AF.Abs)
            ex = sb.tile([1, G], DT.float32, tag="ex")
            nc.scalar.activation(ex[:], ax[:], AF.Exp, scale=-1.0)
            lx = sb.tile([1, G], DT.float32, tag="lx")
            nc.scalar.activation(lx[:], ex[:], AF.Ln, bias=1.0)
            rx = sb.tile([1, G], DT.float32, tag="rx")
            nc.scalar.activation(rx[:], x_sb[:], AF.Relu)
            oi = sb.tile([1, G], DT.float32, tag="oi2")
            nc.vector.tensor_tensor(oi[:], lx[:], rx[:], ALU.add)
            nc.sync.dma_start(out_ext.ap().rearrange("g x -> x g"), oi[:])

    nc.compile()
    return nc


class SpmdRunner:
    def __init__(self, nc, n_cores=P):
        import jax
        import concourse.mybir as mybir
        from concourse import bass2jax
        from jax.sharding import Mesh, PartitionSpec
        from jax.experimental.shard_map import shard_map

        bass2jax.install_neuronx_cc_hook()
        self.n_cores = n_cores
        in_names, out_names, out_avals, zero_outs = [], [], [], []
        partition_name = nc.partition_id_tensor.name if nc.partition_id_tensor else None
        for alloc in nc.m.functions[0].allocations:
            if not isinstance(alloc, mybir.MemoryLocationSet):
                continue
            name = alloc.memorylocations[0].name
            if alloc.kind == "ExternalInput":
                if name != partition_name:
                    in_names.append(name)
            elif alloc.kind == "ExternalOutput":
                out_names.append(name)
                shape = tuple(alloc.tensor_shape)
                dtype = mybir.dt.np(alloc.dtype)
                out_avals.append(jax.core.ShapedArray(shape, dtype))
                zero_outs.append(np.zeros(shape, dtype))
        self.in_names, self.out_names = in_names, out_names
        self.out_avals, self.zero_outs = out_avals, zero_outs
        n_params, n_outs = len(in_names), len(out_names)
        all_in_names = list(in_names) + list(out_names)
        if partition_name is not None:
            all_in_names.append(partition_name)

        def _body(*args):
            operands = list(args)
            if partition_name is not None:
                operands.append(bass2jax.partition_id_tensor())
            outs = bass2jax._bass_exec_p.bind(
                *operands,
                out_avals=tuple(out_avals),
                in_names=tuple(all_in_names),
                out_names=tuple(out_names),
                lowering_input_output_aliases=(),
                sim_require_finite=True,
                sim_require_nnan=True,
                nc=nc,
            )
            return tuple(outs)

        devices = jax.devices()[:n_cores]
        mesh = Mesh(np.asarray(devices), ("core",))
        in_specs = (PartitionSpec("core"),) * (n_params + n_outs)
        out_specs = (PartitionSpec("core"),) * n_outs
        self.sharded = jax.jit(
            shard_map(_body, mesh=mesh, in_specs=in_specs,
                      out_specs=out_specs, check_rep=False),
            keep_unused=True,
        )
        self._jax = jax

    def prepare(self, in_maps):
        jax = self._jax
        n = self.n_cores
        concat_in = [
            np.concatenate([np.asarray(in_maps[c][name]) for c in range(n)], axis=0)
            for name in self.in_names
        ]
        concat_zeros = [
            np.zeros((n * z.shape[0], *z.shape[1:]), z.dtype) for z in self.zero_outs
        ]
        self.args = [jax.device_put(a) for a in concat_in + concat_zeros]

    def run(self):
        jax = self._jax
        outs = self.sharded(*self.args)
        jax.block_until_ready(outs)
        return [
            {
                name: np.asarray(outs[i]).reshape(self.n_cores, *self.out_avals[i].shape)[c]
                for i, name in enumerate(self.out_names)
            }
            for c in range(self.n_cores)
        ]

    def time_it(self, iters=12, warmup=2):
        import time
        jax = self._jax
        for _ in range(warmup):
            jax.block_until_ready(self.sharded(*self.args))
        times = []
        for _ in range(iters):
            t0 = time.perf_counter()
            jax.block_until_ready(self.sharded(*self.args))
            times.append(time.perf_counter() - t0)
        return min(times), float(np.median(times))


_CACHE = {}


def _get_runner(inputs, nrep=1):
    import zlib
    struct, per_core, consts = build_host(inputs)
    _nonce = (zlib.crc32((repr(struct["seq"]) + "v7" + str(nrep)).encode()) % 997) + 2
    key = (struct["NCH"], nrep)
    if key not in _CACHE:
        nc = build_nc(struct, nrep=nrep)
        _CACHE[key] = SpmdRunner(nc, P)
    runner = _CACHE[key]
    in_maps = []
    for c in range(P):
        m = dict(consts)
        m.update({
            "nonce": np.zeros((1, _nonce), np.float32),
            "idx": per_core[c]["idx"], "S": per_core[c]["S"],
            "lcT": per_core[c]["lcT"], "dis_tile": per_core[c]["dis_tile"],
            "eps_tile": per_core[c]["eps_tile"], "B_tile": per_core[c]["B_tile"],
        })
        in_maps.append(m)
    runner.prepare(in_maps)
    return runner


def kernel(**inputs):
    runner = _get_runner(inputs, nrep=1)
    outs = runner.run()
    return outs[0]["out"].astype(np.float32)


if __name__ == "__main__":
    d = np.load("/root/problem/dev/ref_inputs.npz")
    inputs = {k: d[k] for k in d.files}
    out = kernel(**inputs)
    print(out[:4].ravel())



# revision 23
# speedup vs baseline: 1.9294x; 1.0788x over previous
"""ALCDEF Temporal GNN (gnn_message_passing) on 8 TRN2 NeuronCores.

Self-contained: takes FULL unsharded inputs, returns FULL [64,1] output.

Strategy (all shapes hardcoded for N=50000, E=800000, F_IN=16, H=128, L=3, G=64):
- Nodes dst-sharded across 8 cores (6250 each, padded to 6656 local slots;
  local node l = w*128 + p for window w, partition p).
- Per layer each core computes its shard of the message table
  m~[n] = dis_n * (h @ W_l) (node-major bf16 rows, row r = c*6656 + p*52 + w)
  and AllGathers it so every core holds the full table in DRAM.
- Edge aggregation: SWDGE dma_gather pulls 1024 message rows per call,
  round-robined over 2 SWDGE queues so descriptor generation pipelines
  across Q7 core pairs (queues 2-3 collide with the collective's CC cores
  and deadlock; 2 queues are safe and ~2x faster than 1)
  (edge-major msgT [128 edges, 128 feat]); one-hot S matrices (streamed
  bf16, built on host) scatter-add them into per-window PSUM banks via
  TensorE matmuls; self-loops are injected with an identity matmul from
  the local node-major m~ copy; the GCN bias rides a dedicated table row
  with S value 1/dis_d.
- LayerNorm is computed node-major per PSUM bank with an exact per-node
  eps correction (eps_d = eps*deg_d) that makes the dis_d folding exact.
- Mean-pool via per-window matmuls with a host-built B matrix, AllReduce,
  then the small head MLP (softplus composed from Relu/Exp/Ln) on every
  core; core 0's output is returned.
"""
import sys
sys.path.insert(0, "/opt/trn_rl_repo")
import numpy as np
import ml_dtypes
from contextlib import ExitStack

bf16 = ml_dtypes.bfloat16
f32 = np.float32

N, E, F_IN, H, L, G = 50000, 800000, 16, 128, 3, 64
LN_EPS = 1e-5
P = 8
SH = 6656
W = 128
NW = SH // W          # 52
BANKS = NW // 4       # 13
SHARD = 6250
HALF = 4 * SH         # 26624 table rows per half
BATCH = 8             # chunks per gather call (1024 idxs)
NQ = 2                # SWDGE queues (desc-gen pipelines across Q7 core pairs)
NT512 = SH // 512     # 13


def _row_of_local(l):
    w, p = l // W, l % W
    return p * NW + w


def build_host(inputs):
    edge_index = np.asarray(inputs["edge_index"])
    batch = np.asarray(inputs["batch"]).astype(np.int64)
    src_g = edge_index[0].astype(np.int64)
    dst_g = edge_index[1].astype(np.int64)

    deg = np.bincount(dst_g, minlength=N).astype(np.float64) + 1.0
    dis = (1.0 / np.sqrt(deg)).astype(np.float64)

    own = np.minimum(src_g // SHARD, P - 1)
    src_loc = src_g - own * SHARD
    src_row = own * SH + (src_loc % W) * NW + (src_loc // W)

    own_d = np.minimum(dst_g // SHARD, P - 1)
    d_loc = dst_g - own_d * SHARD
    win = d_loc // W
    half = (src_row >= HALF).astype(np.int64)

    BROW = _row_of_local(6250)
    ZROW = _row_of_local(6251)

    # one gather slot per distinct (window, half, src); S row holds all its
    # dst columns (values = edge multiplicity)
    slot_cnt = np.zeros((P, NW, 2), dtype=np.int64)
    for c in range(P):
        m = own_d == c
        for h in range(2):
            mm_ = m & (half == h)
            pairs = win[mm_] * (HALF + 1) + src_row[mm_] - h * HALF
            slot_cnt[c, :, h] = np.bincount(
                np.unique(pairs) // (HALF + 1), minlength=NW)
    slot_cnt[:, :, 0] += 1  # bias slot in half A
    chunks_max = np.ceil(slot_cnt / 128).astype(np.int64).max(axis=0)

    seq = []  # (bank, half, wi, start)
    for b in range(BANKS):
        for h in range(2):
            for wi in range(4):
                for k in range(chunks_max[4 * b + wi, h]):
                    seq.append((b, h, wi, k == 0 and h == 0))
    NCH = len(seq)
    struct = {"seq": seq, "NCH": NCH}

    per_core = []
    lc = np.asarray(inputs["lightcurve"], f32)
    counts = np.bincount(batch, minlength=G).astype(np.float64)
    for c in range(P):
        m = np.flatnonzero(own_d == c)
        e_half = half[m]
        e_win = win[m]
        e_row = src_row[m] - e_half * HALF
        e_col = d_loc[m] % W

        idx_blob = np.zeros((NCH, 128), dtype=np.int16)
        S = np.zeros((NCH, 128, W), dtype=f32)

        pos = {}
        for k, (b, h, wi, st) in enumerate(seq):
            if wi >= 0:
                pos.setdefault((4 * b + wi, h), []).append(k)
        for (w_, h), ks in pos.items():
            sel = np.flatnonzero((e_win == w_) & (e_half == h))
            rows_e = e_row[sel]
            cols_e = e_col[sel]
            uniq, inv = np.unique(rows_e, return_inverse=True)
            nslots = len(uniq)
            cap = len(ks) * 128
            bias_slot = nslots if h == 0 else -1
            total = nslots + (1 if h == 0 else 0)
            assert total <= cap, (w_, h, total, cap)
            slot_rows = np.full(cap, ZROW, dtype=np.int16)
            slot_rows[:nslots] = uniq.astype(np.int16)
            if h == 0:
                slot_rows[bias_slot] = BROW
            for j, k in enumerate(ks):
                idx_blob[k] = slot_rows[j * 128:(j + 1) * 128]
            kk = np.array(ks)[inv // 128]
            np.add.at(S, (kk, inv % 128, cols_e), 1.0)
            if h == 0:
                kb = ks[bias_slot // 128]
                node0 = w_ * W
                g0 = c * SHARD
                nreal = min(W, SHARD - node0)
                if nreal > 0:
                    S[kb, bias_slot % 128, :nreal] = \
                        1.0 / dis[g0 + node0:g0 + node0 + nreal]

        wrapped = idx_blob.reshape(NCH, 8, 16).transpose(2, 0, 1).reshape(16, NCH * 8)
        wrapped = np.tile(wrapped, (8, 1)).astype(np.int16)

        g0 = c * SHARD
        dis_loc = np.zeros(SH, f32)
        dis_loc[:SHARD] = dis[g0:g0 + SHARD]
        eps_loc = np.full(SH, LN_EPS, f32)
        eps_loc[:SHARD] = (LN_EPS * deg[g0:g0 + SHARD]).astype(f32)
        dis_tile = dis_loc.reshape(NW, W).T.copy()
        eps_tile = eps_loc.reshape(NW, W).T.copy()

        Bp = np.zeros((SH, G), f32)
        bb = batch[g0:g0 + SHARD]
        Bp[np.arange(SHARD), bb] = (1.0 / np.maximum(counts[bb], 1.0)).astype(f32)
        B_tile = Bp.reshape(NW, W, G).transpose(1, 0, 2).copy()

        lcT = np.zeros((F_IN, SH), f32)
        lcT[:, :SHARD] = lc[g0:g0 + SHARD].T

        per_core.append({
            "idx": np.ascontiguousarray(wrapped),
            "S": np.ascontiguousarray(S.transpose(1, 0, 2)).astype(bf16),
            "dis_tile": dis_tile, "eps_tile": eps_tile,
            "B_tile": np.ascontiguousarray(B_tile),
            "lcT": lcT,
        })

    consts = {
        "enc_W1": np.asarray(inputs["enc_W1"], f32),
        "enc_b1": np.asarray(inputs["enc_b1"], f32).reshape(H, 1),
        "enc_W2": np.asarray(inputs["enc_W2"], f32),
        "enc_b2": np.asarray(inputs["enc_b2"], f32).reshape(H, 1),
        "convW": np.asarray(inputs["conv_W"], f32).astype(bf16),
        "conv_b_bf": np.asarray(inputs["conv_b"], f32).astype(bf16).reshape(L, 1, H),
        "g_rep": np.tile(np.asarray(inputs["ln_g"], f32)[:, None, :], (1, 128, 1)),
        "b_rep": np.tile(np.asarray(inputs["ln_b"], f32)[:, None, :], (1, 128, 1)),
        "ident_bf": np.eye(128, dtype=f32).astype(bf16),
        "ident_f32": np.eye(128, dtype=f32),
        "hW1": np.asarray(inputs["h_W1"], f32),
        "hb1": np.asarray(inputs["h_b1"], f32).reshape(64, 1),
        "hW2": np.asarray(inputs["h_W2"], f32),
        "hb2": np.asarray(inputs["h_b2"], f32).reshape(32, 1),
        "hW3": np.asarray(inputs["h_W3"], f32),
        "hb3": np.asarray(inputs["h_b3"], f32).reshape(1, 1),
    }
    return struct, per_core, consts


def build_nc(struct, nrep=1):
    import zlib
    nonce = (zlib.crc32((repr(struct["seq"]) + "v8" + str(nrep)).encode()) % 997) + 2
    import concourse.bass as bass
    import concourse.bacc as bacc
    import concourse.mybir as mybir
    import concourse.tile as tile
    from concourse.library_config import mlp as mlp_lib

    seq = struct["seq"]
    NCH = struct["NCH"]
    AF = mybir.ActivationFunctionType
    ALU = mybir.AluOpType
    DT = mybir.dt

    nc = bacc.Bacc("TRN2", debug=False, num_devices=P, num_swdge_queues=NQ)
    dp = nc.declare_dram_parameter
    idx_ext = dp("idx", [128, NCH * 8], DT.int16, isOutput=False)
    S_ext = dp("S", [128, NCH, W], DT.bfloat16, isOutput=False)
    lcT_ext = dp("lcT", [F_IN, SH], DT.float32, isOutput=False)
    dis_ext = dp("dis_tile", [128, NW], DT.float32, isOutput=False)
    eps_ext = dp("eps_tile", [128, NW], DT.float32, isOutput=False)
    B_ext = dp("B_tile", [128, NW, G], DT.float32, isOutput=False)
    encW1_ext = dp("enc_W1", [F_IN, H], DT.float32, isOutput=False)
    encb1_ext = dp("enc_b1", [H, 1], DT.float32, isOutput=False)
    encW2_ext = dp("enc_W2", [H, H], DT.float32, isOutput=False)
    encb2_ext = dp("enc_b2", [H, 1], DT.float32, isOutput=False)
    convW_ext = dp("convW", [L, H, H], DT.bfloat16, isOutput=False)
    convb_ext = dp("conv_b_bf", [L, 1, H], DT.bfloat16, isOutput=False)
    grep_ext = dp("g_rep", [L, 128, H], DT.float32, isOutput=False)
    brep_ext = dp("b_rep", [L, 128, H], DT.float32, isOutput=False)
    identbf_ext = dp("ident_bf", [128, 128], DT.bfloat16, isOutput=False)
    identf_ext = dp("ident_f32", [128, 128], DT.float32, isOutput=False)
    hW1_ext = dp("hW1", [H, 64], DT.float32, isOutput=False)
    hb1_ext = dp("hb1", [64, 1], DT.float32, isOutput=False)
    hW2_ext = dp("hW2", [64, 32], DT.float32, isOutput=False)
    hb2_ext = dp("hb2", [32, 1], DT.float32, isOutput=False)
    hW3_ext = dp("hW3", [32, 1], DT.float32, isOutput=False)
    hb3_ext = dp("hb3", [1, 1], DT.float32, isOutput=False)
    out_ext = dp("out", [G, 1], DT.float32, isOutput=True)
    dp("nonce", [1, nonce], DT.float32, isOutput=False)

    mloc_th = nc.dram_tensor("mloc_i", [128, NW * H], DT.bfloat16)
    table_th = nc.dram_tensor("table_i", [P * SH, H], DT.bfloat16, addr_space="Shared")
    pool_loc_th = nc.dram_tensor("pool_loc_i", [G, H], DT.float32)
    pool_full_th = nc.dram_tensor("pool_full_i", [G, H], DT.float32, addr_space="Shared")

    with tile.TileContext(nc) as tc, ExitStack() as ctx:
        mloc_d = mloc_th.ap()
        table_d = table_th.ap()
        pool_loc_d = pool_loc_th.ap()
        pool_full_d = pool_full_th.ap()
        nc.gpsimd.load_library(mlp_lib)
        pers = ctx.enter_context(tc.tile_pool(name="pers", bufs=1))
        sb = ctx.enter_context(tc.tile_pool(name="sb", bufs=3))
        sstream = ctx.enter_context(tc.tile_pool(name="sstream", bufs=7))
        msgs = ctx.enter_context(tc.tile_pool(name="msgs", bufs=9))
        psA = ctx.enter_context(tc.tile_pool(name="psA", bufs=4, space="PSUM"))
        psB = ctx.enter_context(tc.tile_pool(name="psB", bufs=2, space="PSUM"))
        psC = ctx.enter_context(tc.tile_pool(name="psC", bufs=2, space="PSUM"))

        idx_t = pers.tile([128, NCH * 8], DT.int16, tag="idx")
        nc.sync.dma_start(idx_t[:], idx_ext[:, :])
        dis_t = pers.tile([128, NW], DT.float32, tag="dis")
        nc.sync.dma_start(dis_t[:], dis_ext[:, :])
        eps_t = pers.tile([128, NW], DT.float32, tag="eps")
        nc.sync.dma_start(eps_t[:], eps_ext[:, :])
        B_t = pers.tile([128, NW, G], DT.float32, tag="B")
        nc.sync.dma_start(B_t[:], B_ext[:, :, :])
        lcT_t = pers.tile([F_IN, SH], DT.float32, tag="lcT")
        nc.sync.dma_start(lcT_t[:], lcT_ext[:, :])
        encW1_t = pers.tile([F_IN, H], DT.float32, tag="encW1")
        nc.sync.dma_start(encW1_t[:], encW1_ext[:, :])
        encb1_t = pers.tile([H, 1], DT.float32, tag="encb1")
        nc.sync.dma_start(encb1_t[:], encb1_ext[:, :])
        encW2_t = pers.tile([H, H], DT.float32, tag="encW2")
        nc.sync.dma_start(encW2_t[:], encW2_ext[:, :])
        encb2_t = pers.tile([H, 1], DT.float32, tag="encb2")
        nc.sync.dma_start(encb2_t[:], encb2_ext[:, :])
        convW_t = pers.tile([H, L, H], DT.bfloat16, tag="convW")
        nc.sync.dma_start(convW_t[:], convW_ext.ap().rearrange("l a b -> a l b"))
        convb_t = pers.tile([1, L, H], DT.bfloat16, tag="convb")
        nc.sync.dma_start(convb_t[:], convb_ext.ap().rearrange("l a b -> a l b"))
        grep_t = pers.tile([128, L, H], DT.float32, tag="grep")
        nc.sync.dma_start(grep_t[:], grep_ext.ap().rearrange("l p h -> p l h"))
        brep_t = pers.tile([128, L, H], DT.float32, tag="brep")
        nc.sync.dma_start(brep_t[:], brep_ext.ap().rearrange("l p h -> p l h"))
        identbf_t = pers.tile([128, 128], DT.bfloat16, tag="identbf")
        nc.sync.dma_start(identbf_t[:], identbf_ext[:, :])
        identf_t = pers.tile([128, 128], DT.float32, tag="identf")
        nc.sync.dma_start(identf_t[:], identf_ext[:, :])
        hW1_t = pers.tile([H, 64], DT.float32, tag="hW1")
        nc.sync.dma_start(hW1_t[:], hW1_ext[:, :])
        hb1_t = pers.tile([64, 1], DT.float32, tag="hb1")
        nc.sync.dma_start(hb1_t[:], hb1_ext[:, :])
        hW2_t = pers.tile([64, 32], DT.float32, tag="hW2")
        nc.sync.dma_start(hW2_t[:], hW2_ext[:, :])
        hb2_t = pers.tile([32, 1], DT.float32, tag="hb2")
        nc.sync.dma_start(hb2_t[:], hb2_ext[:, :])
        hW3_t = pers.tile([32, 1], DT.float32, tag="hW3")
        nc.sync.dma_start(hW3_t[:], hW3_ext[:, :])
        hb3_t = pers.tile([1, 1], DT.float32, tag="hb3")
        nc.sync.dma_start(hb3_t[:], hb3_ext[:, :])

        recipH_t = pers.tile([128, 4], DT.float32, tag="recipH")
        nc.vector.memset(recipH_t[:], 1.0 / H)
        hA = pers.tile([128, NW, H], DT.float32, tag="hA")
        hB = pers.tile([128, NW, H], DT.float32, tag="hB")
        hTfm = pers.tile([128, NW, H], DT.bfloat16, tag="hTfm")
        mnm = pers.tile([128, NW, H], DT.bfloat16, tag="mnm")

        for rep in range(nrep):
            # ---- encoder (feature-major) ----
            for t in range(NT512):
                z_ps = psB.tile([128, 512], DT.float32, tag="mm")
                nc.tensor.matmul(z_ps[:], lhsT=encW1_t[:],
                                 rhs=lcT_t[:, t * 512:(t + 1) * 512],
                                 start=True, stop=True)
                z_sb = sb.tile([128, 512], DT.float32, tag="zenc")
                nc.scalar.activation(z_sb[:], z_ps[:], AF.Relu, bias=encb1_t[:, 0:1])
                h0_ps = psC.tile([128, 512], DT.float32, tag="tr")
                nc.tensor.matmul(h0_ps[:], lhsT=encW2_t[:], rhs=z_sb[:],
                                 start=True, stop=True)
                hTv = hTfm[:].rearrange("p w h -> p (w h)")
                nc.scalar.activation(hTv[:, t * 512:(t + 1) * 512], h0_ps[:],
                                     AF.Identity, bias=encb2_t[:, 0:1])
            hTv = hTfm[:].rearrange("p w h -> p (w h)")
            nc.vector.memset(hTv[:, SHARD:SH], 0.0)

            h_in, h_out = hA, hB
            for l in range(L):
                # ---- m~ table path ----
                for t in range(NT512):
                    m_ps = psB.tile([128, 512], DT.float32, tag="mm")
                    nc.tensor.matmul(
                        m_ps[:], lhsT=convW_t[:, l, :],
                        rhs=hTfm[:].rearrange("p w h -> p (w h)")[:, t * 512:(t + 1) * 512],
                        start=True, stop=True)
                    m_fm = sb.tile([128, 512], DT.bfloat16, tag="mfm")
                    nc.vector.tensor_copy(m_fm[:], m_ps[:])
                    for j in range(4):
                        w_ = 4 * t + j
                        mT_ps = psC.tile([128, 128], DT.bfloat16, tag="tr")
                        nc.tensor.transpose(mT_ps[:], m_fm[:, j * 128:(j + 1) * 128],
                                            identbf_t[:])
                        nc.vector.tensor_scalar(
                            out=mnm[:, w_, :], in0=mT_ps[:],
                            scalar1=dis_t[:, w_:w_ + 1], scalar2=None,
                            op0=ALU.mult)
                nc.sync.dma_start(mloc_d, mnm[:].rearrange("p w h -> p (w h)"))
                nc.sync.dma_start(mloc_d[106:107, 48 * H:49 * H], convb_t[:, l, :])
                nc.gpsimd.collective_compute(
                    "AllGather", ALU.bypass,
                    ins=[mloc_d.opt()], outs=[table_d.opt()],
                    replica_groups=[list(range(P))],
                )

                # ---- edge aggregation ----
                gq = 0
                for b in range(BANKS):
                    aggw = [psA.tile([128, 128], DT.float32, tag="aggw",
                                     name=f"aggw_{rep}_{l}_{b}_{_wi}")
                            for _wi in range(4)]
                    for h_sel in range(2):
                        ks = [k for k, s_ in enumerate(seq)
                              if s_[0] == b and s_[1] == h_sel]
                        for bi in range(0, len(ks), BATCH):
                            nck = min(BATCH, len(ks) - bi)
                            k0 = ks[bi]
                            msg = msgs.tile([128, BATCH, H], DT.bfloat16, tag="msg")
                            s_t = sstream.tile([128, BATCH, W], DT.bfloat16, tag="S")
                            nc.sync.dma_start(s_t[:, 0:nck, :], S_ext[:, k0:k0 + nck, :])
                            base = h_sel * HALF
                            nc.gpsimd.dma_gather(
                                msg[:, 0:nck, :], table_d[base:base + HALF, :],
                                idx_t[:, k0 * 8:(k0 + nck) * 8],
                                nck * 128, nck * 128, H,
                                single_packet=False,
                                queue_num=gq % NQ,
                            )
                            gq += 1
                            for j in range(nck):
                                k = k0 + j
                                _, h_, wi_, st_ = seq[k]
                                nc.tensor.matmul(
                                    aggw[wi_][:], lhsT=s_t[:, j, :], rhs=msg[:, j, :],
                                    start=st_, stop=False, skip_group_check=True)
                    for wi in range(4):
                        w_ = 4 * b + wi
                        nc.tensor.matmul(aggw[wi][:], lhsT=identbf_t[:],
                                         rhs=mnm[:, w_, :], start=False, stop=True,
                                         skip_group_check=True)
                    agg = sb.tile([128, 4, 128], DT.float32, tag="aggsb")
                    for wi in range(4):
                        nc.vector.tensor_copy(agg[:, wi, :], aggw[wi][:])
                    # ---- LayerNorm ----
                    st1 = sb.tile([128, 4], DT.float32, tag="st1")
                    st2 = sb.tile([128, 4], DT.float32, tag="st2")
                    sq = sb.tile([128, 4, 128], DT.float32, tag="sq")
                    nc.vector.tensor_reduce(st1[:], agg[:], mybir.AxisListType.X, ALU.add)
                    nc.vector.tensor_tensor(sq[:], agg[:], agg[:], ALU.mult)
                    nc.vector.tensor_reduce(st2[:], sq[:], mybir.AxisListType.X, ALU.add)
                    mu = sb.tile([128, 4], DT.float32, tag="mu")
                    nc.vector.tensor_tensor(mu[:], st1[:], recipH_t[:], ALU.mult)
                    var = sb.tile([128, 4], DT.float32, tag="var")
                    nc.vector.tensor_tensor(var[:], st2[:], recipH_t[:], ALU.mult)
                    mu2 = sb.tile([128, 4], DT.float32, tag="mu2")
                    nc.vector.tensor_tensor(mu2[:], mu[:], mu[:], ALU.mult)
                    nc.vector.tensor_tensor(var[:], var[:], mu2[:], ALU.subtract)
                    nc.vector.tensor_tensor(var[:], var[:], eps_t[:, 4 * b:4 * b + 4],
                                            ALU.add)
                    std = sb.tile([128, 4], DT.float32, tag="std")
                    nc.scalar.sqrt(std[:], var[:])
                    rstd = sb.tile([128, 4], DT.float32, tag="rstd")
                    nc.vector.reciprocal(rstd[:], std[:])
                    tmp = sb.tile([128, 4, 128], DT.float32, tag="tmp")
                    nc.vector.tensor_tensor(
                        tmp[:], agg[:], mu[:, :, None].broadcast_to([128, 4, 128]),
                        ALU.subtract)
                    nc.vector.tensor_tensor(
                        tmp[:], tmp[:], rstd[:, :, None].broadcast_to([128, 4, 128]),
                        ALU.mult)
                    nc.vector.tensor_tensor(
                        tmp[:], tmp[:],
                        grep_t[:, l, :][:, None, :].broadcast_to([128, 4, 128]),
                        ALU.mult)
                    nc.vector.tensor_tensor(
                        tmp[:], tmp[:],
                        brep_t[:, l, :][:, None, :].broadcast_to([128, 4, 128]),
                        ALU.add)
                    if l > 0:
                        nc.vector.tensor_scalar(out=tmp[:], in0=tmp[:], scalar1=0.0,
                                                scalar2=None, op0=ALU.max)
                        nc.vector.tensor_tensor(h_out[:, 4 * b:4 * b + 4, :], tmp[:],
                                                h_in[:, 4 * b:4 * b + 4, :], ALU.add)
                    else:
                        nc.vector.tensor_scalar(out=h_out[:, 4 * b:4 * b + 4, :],
                                                in0=tmp[:], scalar1=0.0,
                                                scalar2=None, op0=ALU.max)

                if l < L - 1:
                    for w_ in range(NW):
                        hT_ps = psC.tile([128, 128], DT.float32, tag="tr")
                        nc.tensor.transpose(hT_ps[:], h_out[:, w_, :], identf_t[:])
                        nc.vector.tensor_copy(hTfm[:, w_, :], hT_ps[:])
                h_in, h_out = h_out, h_in

            h_fin = h_in
            # ---- pooling ----
            pool_ps = psB.tile([G, H], DT.float32, tag="mm")
            for w_ in range(NW):
                nc.tensor.matmul(pool_ps[:], lhsT=B_t[:, w_, :], rhs=h_fin[:, w_, :],
                                 start=(w_ == 0), stop=(w_ == NW - 1))
            pool_sb = sb.tile([G, H], DT.float32, tag="pool")
            nc.vector.tensor_copy(pool_sb[:], pool_ps[:])
            nc.sync.dma_start(pool_loc_d, pool_sb[:])
            nc.gpsimd.collective_compute(
                "AllReduce", ALU.add,
                ins=[pool_loc_d.opt()], outs=[pool_full_d.opt()],
                replica_groups=[list(range(P))],
            )
            poolf = sb.tile([G, H], DT.float32, tag="poolf")
            nc.sync.dma_start(poolf[:], pool_full_d)
            # ---- head ----
            poolT_ps = psC.tile([128, G], DT.float32, tag="tr")
            nc.tensor.transpose(poolT_ps[:], poolf[:], identf_t[:G, :G])
            poolT = sb.tile([128, G], DT.float32, tag="poolT")
            nc.vector.tensor_copy(poolT[:], poolT_ps[:])
            z1_ps = psB.tile([64, G], DT.float32, tag="mm")
            nc.tensor.matmul(z1_ps[:], lhsT=hW1_t[:], rhs=poolT[:], start=True, stop=True)
            z1 = sb.tile([64, G], DT.float32, tag="z1")
            nc.scalar.activation(z1[:], z1_ps[:], AF.Relu, bias=hb1_t[:, 0:1])
            z2_ps = psB.tile([32, G], DT.float32, tag="mm")
            nc.tensor.matmul(z2_ps[:], lhsT=hW2_t[:], rhs=z1[:], start=True, stop=True)
            z2 = sb.tile([32, G], DT.float32, tag="z2")
            nc.scalar.activation(z2[:], z2_ps[:], AF.Relu, bias=hb2_t[:, 0:1])
            z3_ps = psB.tile([1, G], DT.float32, tag="mm")
            nc.tensor.matmul(z3_ps[:], lhsT=hW3_t[:], rhs=z2[:], start=True, stop=True)
            # softplus(x) = relu(x) + ln(1 + exp(-|x|))
            x_sb = sb.tile([1, G], DT.float32, tag="oi")
            nc.scalar.activation(x_sb[:], z3_ps[:], AF.Identity, bias=hb3_t[:, 0:1])
            ax = sb.tile([1, G], DT.float32, tag="ax")
            nc.scalar.activation(ax[:], x_sb[:], AF.Abs)
            ex = sb.tile([1, G], DT.float32, tag="ex")
            nc.scalar.activation(ex[:], ax[:], AF.Exp, scale=-1.0)
            lx = sb.tile([1, G], DT.float32, tag="lx")
            nc.scalar.activation(lx[:], ex[:], AF.Ln, bias=1.0)
            rx = sb.tile([1, G], DT.float32, tag="rx")
            nc.scalar.activation(rx[:], x_sb[:], AF.Relu)
            oi = sb.tile([1, G], DT.float32, tag="oi2")
            nc.vector.tensor_tensor(oi[:], lx[:], rx[:], ALU.add)
            nc.sync.dma_start(out_ext.ap().rearrange("g x -> x g"), oi[:])

    nc.compile()
    return nc


class SpmdRunner:
    def __init__(self, nc, n_cores=P):
        import jax
        import concourse.mybir as mybir
        from concourse import bass2jax
        from jax.sharding import Mesh, PartitionSpec
        from jax.experimental.shard_map import shard_map

        bass2jax.install_neuronx_cc_hook()
        self.n_cores = n_cores
        in_names, out_names, out_avals, zero_outs = [], [], [], []
        partition_name = nc.partition_id_tensor.name if nc.partition_id_tensor else None
        for alloc in nc.m.functions[0].allocations:
            if not isinstance(alloc, mybir.MemoryLocationSet):
                continue
            name = alloc.memorylocations[0].name
            if alloc.kind == "ExternalInput":
                if name != partition_name:
                    in_names.append(name)
            elif alloc.kind == "ExternalOutput":
                out_names.append(name)
                shape = tuple(alloc.tensor_shape)
                dtype = mybir.dt.np(alloc.dtype)
                out_avals.append(jax.core.ShapedArray(shape, dtype))
                zero_outs.append(np.zeros(shape, dtype))
        self.in_names, self.out_names = in_names, out_names
        self.out_avals, self.zero_outs = out_avals, zero_outs
        n_params, n_outs = len(in_names), len(out_names)
        all_in_names = list(in_names) + list(out_names)
        if partition_name is not None:
            all_in_names.append(partition_name)

        def _body(*args):
            operands = list(args)
            if partition_name is not None:
                operands.append(bass2jax.partition_id_tensor())
            outs = bass2jax._bass_exec_p.bind(
                *operands,
                out_avals=tuple(out_avals),
                in_names=tuple(all_in_names),
                out_names=tuple(out_names),
                lowering_input_output_aliases=(),
                sim_require_finite=True,
                sim_require_nnan=True,
                nc=nc,
            )
            return tuple(outs)

        devices = jax.devices()[:n_cores]
        mesh = Mesh(np.asarray(devices), ("core",))
        in_specs = (PartitionSpec("core"),) * (n_params + n_outs)
        out_specs = (PartitionSpec("core"),) * n_outs
        self.sharded = jax.jit(
            shard_map(_body, mesh=mesh, in_specs=in_specs,
                      out_specs=out_specs, check_rep=False),
            keep_unused=True,
        )
        self._jax = jax

    def prepare(self, in_maps):
        jax = self._jax
        n = self.n_cores
        concat_in = [
            np.concatenate([np.asarray(in_maps[c][name]) for c in range(n)], axis=0)
            for name in self.in_names
        ]
        concat_zeros = [
            np.zeros((n * z.shape[0], *z.shape[1:]), z.dtype) for z in self.zero_outs
        ]
        self.args = [jax.device_put(a) for a in concat_in + concat_zeros]

    def run(self):
        jax = self._jax
        outs = self.sharded(*self.args)
        jax.block_until_ready(outs)
        return [
            {
                name: np.asarray(outs[i]).reshape(self.n_cores, *self.out_avals[i].shape)[c]
                for i, name in enumerate(self.out_names)
            }
            for c in range(self.n_cores)
        ]

    def time_it(self, iters=12, warmup=2):
        import time
        jax = self._jax
        for _ in range(warmup):
            jax.block_until_ready(self.sharded(*self.args))
        times = []
        for _ in range(iters):
            t0 = time.perf_counter()
            jax.block_until_ready(self.sharded(*self.args))
            times.append(time.perf_counter() - t0)
        return min(times), float(np.median(times))


_CACHE = {}


def _get_runner(inputs, nrep=1):
    import zlib
    struct, per_core, consts = build_host(inputs)
    _nonce = (zlib.crc32((repr(struct["seq"]) + "v8" + str(nrep)).encode()) % 997) + 2
    key = (struct["NCH"], nrep)
    if key not in _CACHE:
        nc = build_nc(struct, nrep=nrep)
        _CACHE[key] = SpmdRunner(nc, P)
    runner = _CACHE[key]
    in_maps = []
    for c in range(P):
        m = dict(consts)
        m.update({
            "nonce": np.zeros((1, _nonce), np.float32),
            "idx": per_core[c]["idx"], "S": per_core[c]["S"],
            "lcT": per_core[c]["lcT"], "dis_tile": per_core[c]["dis_tile"],
            "eps_tile": per_core[c]["eps_tile"], "B_tile": per_core[c]["B_tile"],
        })
        in_maps.append(m)
    runner.prepare(in_maps)
    return runner


def kernel(**inputs):
    runner = _get_runner(inputs, nrep=1)
    outs = runner.run()
    return outs[0]["out"].astype(np.float32)


if __name__ == "__main__":
    d = np.load("/root/problem/dev/ref_inputs.npz")
    inputs = {k: d[k] for k in d.files}
    out = kernel(**inputs)
    print(out[:4].ravel())

